# revision 8
# baseline (speedup 1.0000x reference)
"""Belief-propagation (segment-logsumexp message passing) on 8 TRN2 cores.

Strategy:
  Phase 1 is segment-sharded: core c owns segments [c*NB/8, (c+1)*NB/8).
  Host builds CSR-style member-offset tables (pure int index marshaling);
  the device gathers every member row of its segments from the full
  theta_a via indirect DMA (128 rows per instruction), computes
  exp -> per-segment tree reduction -> log, adds theta_b rows -> tb_un.
  Since logsumexp(ta) == logsumexp(tb) exactly (junction-tree partition
  function), the global normalizer Z needs only a [128,1] AllReduce of
  per-core sums of exp(tb_un).
  Phase 2 is row-sharded: core c streams theta_a rows [c*NA/8, ...),
  gathers theta_b[idx] per 128 rows, adds, subtracts Z, writes ta.

Outputs are assembled on host by concatenation only.
"""
import os
import numpy as np

import concourse.bacc as bacc
import concourse.bass as bass
import concourse.mybir as mybir
import concourse.tile as tile

F32 = mybir.dt.float32
I32 = mybir.dt.int32
NEG = -80.0  # pad log-potential: exp(-80) ~ 2e-35, negligible but finite


def _structure(idx_map, na, nb, n_cores):
    """Host-side index marshaling: class-padded member tables per core."""
    counts = np.bincount(idx_map, minlength=nb).astype(np.int64)
    order = np.argsort(idx_map, kind="stable").astype(np.int64)
    starts = np.zeros(nb + 1, np.int64)
    starts[1:] = np.cumsum(counts)

    spc = nb // n_cores                      # segments per core
    # class of a segment: member slots padded to a multiple of 4 (min 4)
    kls = np.maximum(4, ((counts + 3) // 4) * 4).astype(np.int64)
    class_vals = sorted(set(kls.tolist()))

    # per class: max segment count over cores, padded to a multiple of 128
    n_per_class = {}
    for kv in class_vals:
        mx = 0
        for c in range(n_cores):
            mx = max(mx, int((kls[c * spc:(c + 1) * spc] == kv).sum()))
        n_per_class[kv] = ((mx + 127) // 128) * 128

    # block schedule shared by all cores: list of (K, #blocks)
    sched = [(kv, n_per_class[kv] // 128) for kv in class_vals if n_per_class[kv] > 0]
    nblk = sum(nb_ for _, nb_ in sched)
    ncall1 = sum(kv * nb_ for kv, nb_ in sched)

    mem_offs = np.full((n_cores, 128, ncall1), na, np.int32)     # pad -> NEG row
    tbrow = np.zeros((n_cores, 128, nblk), np.int32)
    tbout = np.full((n_cores, 128, nblk), spc, np.int32)         # dummy -> trash row

    for c in range(n_cores):
        segs = np.arange(c * spc, (c + 1) * spc)
        k_c = kls[segs]
        call0 = 0
        blk0 = 0
        for kv, nblks in sched:
            cls_segs = segs[k_c == kv]
            for j, s in enumerate(cls_segs):
                p = j % 128
                b = blk0 + j // 128
                tbrow[c, p, b] = s
                tbout[c, p, b] = s - c * spc
                m = order[starts[s]:starts[s + 1]]
                base = call0 + (j // 128) * kv
                mem_offs[c, p, base:base + len(m)] = m
            call0 += kv * nblks
            blk0 += nblks
    return sched, nblk, ncall1, mem_offs, tbrow, tbout


def _build(na, nb, n_cores, sched, nblk, ncall1, g2, repeat=1):
    spc = nb // n_cores
    rpc = na // n_cores                      # rows per core
    nc2 = rpc // 128                         # phase-2 gather calls
    nt2 = rpc // (128 * g2)                  # phase-2 tiles

    nc = bacc.Bacc(None, target_bir_lowering=False, debug=False,
                   num_devices=n_cores, dynamic_dma_scratch_size=16384)

    ta_full = nc.dram_tensor("ta_full", [na + 1, 32], F32, kind="ExternalInput")
    ta_slice = nc.dram_tensor("ta_slice", [rpc, 32], F32, kind="ExternalInput")
    tb_full = nc.dram_tensor("tb_full", [nb, 32], F32, kind="ExternalInput")
    mem_o = nc.dram_tensor("mem_o", [128, ncall1], I32, kind="ExternalInput")
    tbrow_o = nc.dram_tensor("tbrow_o", [128, nblk], I32, kind="ExternalInput")
    tbout_o = nc.dram_tensor("tbout_o", [128, nblk], I32, kind="ExternalInput")
    p2_o = nc.dram_tensor("p2_o", [128, nc2], I32, kind="ExternalInput")

    ta_part = nc.dram_tensor("ta_part", [rpc, 32], F32, kind="ExternalOutput")
    tb_part = nc.dram_tensor("tb_part", [spc + 1, 32], F32, kind="ExternalOutput")

    ar_in = nc.dram_tensor("ar_in", [128, 1], F32)
    ar_out = nc.dram_tensor("ar_out", [128, 1], F32, addr_space="Shared")

    kmax = max(kv for kv, _ in sched)

    with tile.TileContext(nc) as tc:
        with tc.tile_pool(name="const", bufs=1) as cp, \
             tc.tile_pool(name="work", bufs=3) as wp, \
             tc.tile_pool(name="ps", bufs=1, space="PSUM") as pp:

            memo_t = cp.tile([128, ncall1], I32)
            nc.sync.dma_start(out=memo_t[:], in_=mem_o[:])
            tbrow_t = cp.tile([128, nblk], I32)
            nc.sync.dma_start(out=tbrow_t[:], in_=tbrow_o[:])
            tbout_t = cp.tile([128, nblk], I32)
            nc.sync.dma_start(out=tbout_t[:], in_=tbout_o[:])
            p2_t = cp.tile([128, nc2], I32)
            nc.sync.dma_start(out=p2_t[:], in_=p2_o[:])

            tbu = cp.tile([128, nblk, 32], F32)      # tb_un, held across stages
            scr = cp.tile([128, nblk, 32], F32)
            psum_acc = cp.tile([128, 1], F32)
            ps_all = cp.tile([128, 1], F32)
            ones = cp.tile([128, 128], F32)
            z128 = cp.tile([128, 1], F32)
            tbv = cp.tile([128, nblk, 32], F32)

            for _rep in range(repeat):
                # ---- Phase 1: per-segment gather + reduce ----
                call = 0
                blk = 0
                for kv, nblks in sched:
                    for _ in range(nblks):
                        x = wp.tile([128, kmax, 32], F32, tag="x")
                        for k in range(kv):
                            nc.gpsimd.indirect_dma_start(
                                out=x[:, k, :], out_offset=None, in_=ta_full[:],
                                in_offset=bass.IndirectOffsetOnAxis(
                                    ap=memo_t[:, call:call + 1], axis=0))
                            call += 1
                        e = wp.tile([128, kmax, 32], F32, tag="e")
                        nc.scalar.activation(
                            e[:, 0:kv, :].rearrange("p a b -> p (a b)"),
                            x[:, 0:kv, :].rearrange("p a b -> p (a b)"),
                            mybir.ActivationFunctionType.Exp)
                        # tree-reduce kv slots -> slot 0 (in place on e)
                        w = kv
                        while w > 1:
                            h = w // 2
                            nc.vector.tensor_add(e[:, 0:h, :], e[:, 0:h, :],
                                                 e[:, h:2 * h, :])
                            if w % 2:
                                nc.vector.tensor_add(e[:, 0:1, :], e[:, 0:1, :],
                                                     e[:, w - 1:w, :])
                            w = h
                        b_t = wp.tile([128, 32], F32, tag="b")
                        nc.gpsimd.indirect_dma_start(
                            out=b_t[:], out_offset=None, in_=tb_full[:],
                            in_offset=bass.IndirectOffsetOnAxis(
                                ap=tbrow_t[:, blk:blk + 1], axis=0))
                        ls = wp.tile([128, 32], F32, tag="ls")
                        nc.scalar.activation(ls[:], e[:, 0, :],
                                             mybir.ActivationFunctionType.Ln)
                        nc.vector.tensor_add(tbu[:, blk, :], ls[:], b_t[:])
                        blk += 1

                # ---- partial sum of exp(tb_un) ----
                nc.scalar.activation(scr[:].rearrange("p a b -> p (a b)"),
                                     tbu[:].rearrange("p a b -> p (a b)"),
                                     mybir.ActivationFunctionType.Exp,
                                     accum_out=psum_acc[:])

                # ---- AllReduce + Z ----
                nc.sync.dma_start(out=ar_in[:], in_=psum_acc[:])
                nc.gpsimd.collective_compute(
                    "AllReduce", mybir.AluOpType.add,
                    replica_groups=[list(range(n_cores))],
                    ins=[ar_in[:]], outs=[ar_out[:]])
                nc.sync.dma_start(out=ps_all[:], in_=ar_out[:])
                nc.vector.memset(ones[:], 1.0)
                tot = pp.tile([128, 1], F32)
                nc.tensor.matmul(tot[:], lhsT=ones[:], rhs=ps_all[:],
                                 start=True, stop=True)
                nc.scalar.activation(z128[:], tot[:],
                                     mybir.ActivationFunctionType.Ln)

                # ---- tb output ----
                nc.vector.tensor_scalar(tbv[:].rearrange("p a b -> p (a b)"),
                                        tbu[:].rearrange("p a b -> p (a b)"),
                                        z128[:, 0:1], None,
                                        mybir.AluOpType.subtract)
                for b in range(nblk):
                    nc.gpsimd.indirect_dma_start(
                        out=tb_part[:],
                        out_offset=bass.IndirectOffsetOnAxis(
                            ap=tbout_t[:, b:b + 1], axis=0),
                        in_=tbv[:, b, :], in_offset=None)

                # ---- Phase 2: ta rows ----
                for t in range(nt2):
                    a_t = wp.tile([128, g2, 32], F32, tag="a2")
                    nc.sync.dma_start(
                        out=a_t[:],
                        in_=ta_slice[t * 128 * g2:(t + 1) * 128 * g2, :]
                        .rearrange("(p g) b -> p g b", p=128))
                    m_t = wp.tile([128, g2, 32], F32, tag="m2")
                    for j in range(g2):
                        nc.gpsimd.indirect_dma_start(
                            out=m_t[:, j, :], out_offset=None, in_=tb_full[:],
                            in_offset=bass.IndirectOffsetOnAxis(
                                ap=p2_t[:, t * g2 + j:t * g2 + j + 1], axis=0))
                    o_t = wp.tile([128, g2, 32], F32, tag="o2")
                    nc.vector.tensor_add(o_t[:], a_t[:], m_t[:])
                    nc.vector.tensor_scalar(o_t[:].rearrange("p a b -> p (a b)"),
                                            o_t[:].rearrange("p a b -> p (a b)"),
                                            z128[:, 0:1], None,
                                            mybir.AluOpType.subtract)
                    nc.sync.dma_start(
                        out=ta_part[t * 128 * g2:(t + 1) * 128 * g2, :]
                        .rearrange("(p g) b -> p g b", p=128),
                        in_=o_t[:])

    nc.compile()
    return nc


def _prep_inputs(theta_a, theta_b, idx_map, n_cores, g2):
    na, r = theta_a.shape
    nb = theta_b.shape[0]
    rpc = na // n_cores

    sched, nblk, ncall1, mem_offs, tbrow, tbout = _structure(
        idx_map, na, nb, n_cores)

    ta_full = np.concatenate([theta_a, np.full((1, r), NEG, np.float32)], axis=0)
    idx2 = idx_map.astype(np.int32).reshape(n_cores, rpc // (128 * g2), 128, g2)
    p2 = idx2.transpose(0, 2, 1, 3).reshape(n_cores, 128, rpc // 128)

    in_maps = []
    for c in range(n_cores):
        in_maps.append({
            "ta_full": ta_full,
            "ta_slice": np.ascontiguousarray(theta_a[c * rpc:(c + 1) * rpc]),
            "tb_full": theta_b,
            "mem_o": np.ascontiguousarray(mem_offs[c]),
            "tbrow_o": np.ascontiguousarray(tbrow[c]),
            "tbout_o": np.ascontiguousarray(tbout[c]),
            "p2_o": np.ascontiguousarray(p2[c]),
        })
    return sched, nblk, ncall1, in_maps


def kernel(theta_a, theta_b, idx_map):
    from concourse.bass_utils import run_bass_kernel_spmd

    na, r = theta_a.shape
    nb = theta_b.shape[0]
    n_cores = 8
    g2 = 64
    spc = nb // n_cores

    idx_map = np.asarray(idx_map)
    theta_a = np.asarray(theta_a, dtype=np.float32)
    theta_b = np.asarray(theta_b, dtype=np.float32)

    sched, nblk, ncall1, in_maps = _prep_inputs(theta_a, theta_b, idx_map,
                                                n_cores, g2)
    nc = _build(na, nb, n_cores, sched, nblk, ncall1, g2,
                repeat=int(os.environ.get("BP_REPEAT", "1")))
    res = run_bass_kernel_spmd(nc, in_maps, core_ids=list(range(n_cores)))

    ta = np.concatenate([res.results[c]["ta_part"] for c in range(n_cores)],
                        axis=0)
    tb = np.concatenate([res.results[c]["tb_part"][:spc] for c in range(n_cores)],
                        axis=0)
    return ta, tb
